# Initial kernel scaffold
#
"""TRN2 Bass kernel for nn_Attention_69655779606628 (8-core SPMD).

BN+ReLU / QKV self-attention / softmax / BN+ReLU / residual.

Sharding: data-parallel over batch b=8 -> one batch item per NeuronCore; the
small [256,256] weights and BN params are replicated. BN1 batch stats are
computed on host (x is fully known there and folds to a per-channel affine);
BN2 batch stats (over the attention output) are computed on device and
synchronized across the 8 cores with AllReduces (exact sync-BN semantics) --
split into two collectives so the first one's latency and cross-core skew
hide under the last attention tiles' compute.

All matmuls run as float32r (full-rate fp32 mode of the PE array); softmax is
computed in transposed [k,q] layout so no transposes are needed anywhere.
"""

import sys

for _p in ("/opt/trn_rl_repo", "/root/.axon_site/_ro/trn_rl_repo"):
    if _p not in sys.path:
        sys.path.insert(0, _p)

import numpy as np
from contextlib import ExitStack

import concourse.bass as bass
import concourse.bass_isa as bass_isa
import concourse.mybir as mybir
import concourse.tile as tile
from concourse import bacc
from concourse.bass_utils import run_bass_kernel_spmd

F32 = mybir.dt.float32
F32R = mybir.dt.float32r
AF = mybir.ActivationFunctionType
AX = mybir.AxisListType

EPS = 1e-5
NCORES = 8
C = 256
N = 4096  # h*w = 64*64

def _host_prep(x_all, wq, wk, wv, wo, gq, bq, gk, bk, gv, bv, go, bo):
    """Host-side prep: BN1 stats + per-core input maps."""
    b = x_all.shape[0]
    assert b == NCORES
    xv = x_all.reshape(b, C, N)

    x64 = xv.astype(np.float64)
    mean = x64.mean(axis=(0, 2))
    var = ((x64 - mean[None, :, None]) ** 2).mean(axis=(0, 2))
    inv = 1.0 / np.sqrt(var + EPS)

    def fold(g, bb):
        s = g.astype(np.float64) * inv
        t = bb.astype(np.float64) - mean * s
        return s.astype(np.float32), t.astype(np.float32)

    sq, tq = fold(gq, bq)
    sk, tk = fold(gk, bk)
    sv, tv = fold(gv, bv)
    shared_h = (
        np.allclose(sq, sk) and np.allclose(sq, sv)
        and np.allclose(tq, tk) and np.allclose(tq, tv)
    )

    bn1 = np.stack([sq, tq, sk, tk, sv, tv], axis=1).astype(np.float32)  # [256, 6]
    bn2 = np.stack([go, bo], axis=1).astype(np.float32)  # [256, 2]

    common = {
        "wqT": np.ascontiguousarray(wq.T).astype(np.float32),
        "wkT": np.ascontiguousarray(wk.T).astype(np.float32),
        "wvT": np.ascontiguousarray(wv.T).astype(np.float32),
        "woT": np.ascontiguousarray(wo.T).astype(np.float32),
        "bn1": bn1,
        "bn2": bn2,
    }
    in_maps = [
        {"x": np.ascontiguousarray(xv[i]), **common} for i in range(NCORES)
    ]
    return in_maps, shared_h


def _build(nc: bass.Bass, shared_h: bool):
    n = N
    fake_cc = False
    reps = 1
    assert n % 512 == 0
    NQ = n // 512   # query tiles (512 wide)
    NK = n // 128   # key tiles (128 wide)
    count = float(NCORES * n)  # BN2 element count per channel

    x_d = nc.dram_tensor("x", [C, n], F32, kind="ExternalInput")
    w_d = {
        nm: nc.dram_tensor(nm, [C, C], F32, kind="ExternalInput")
        for nm in ("wqT", "wkT", "wvT", "woT")
    }
    bn1_d = nc.dram_tensor("bn1", [C, 6], F32, kind="ExternalInput")
    bn2_d = nc.dram_tensor("bn2", [C, 2], F32, kind="ExternalInput")
    out_d = nc.dram_tensor("out", [C, n], F32, kind="ExternalOutput")
    cc_in_a = nc.dram_tensor("cc_in_a", [128, 4], F32)
    cc_out_a = nc.dram_tensor("cc_out_a", [128, 4], F32, addr_space="Shared")
    cc_in_b = nc.dram_tensor("cc_in_b", [128, 4], F32)
    cc_out_b = nc.dram_tensor("cc_out_b", [128, 4], F32, addr_space="Shared")

    with tile.TileContext(nc) as tc, ExitStack() as ctx:
        consts = ctx.enter_context(tc.tile_pool(name="consts", bufs=1))
        wraw = ctx.enter_context(tc.tile_pool(name="wraw", bufs=2))
        bigA = ctx.enter_context(tc.tile_pool(name="bigA", bufs=2))  # x -> rT
        bigB = ctx.enter_context(tc.tile_pool(name="bigB", bufs=2))  # h -> x2
        bigC = ctx.enter_context(tc.tile_pool(name="bigC", bufs=2))  # q -> ho
        bigD = ctx.enter_context(tc.tile_pool(name="bigD", bufs=2))  # k
        bigE = ctx.enter_context(tc.tile_pool(name="bigE", bufs=1))  # v
        attn = ctx.enter_context(tc.tile_pool(name="attn", bufs=4))
        smalls = ctx.enter_context(tc.tile_pool(name="smalls", bufs=2))
        statp = ctx.enter_context(tc.tile_pool(name="statp", bufs=1))
        outp = ctx.enter_context(tc.tile_pool(name="outp", bufs=2))
        psA = ctx.enter_context(tc.tile_pool(name="psA", bufs=4, space="PSUM"))
        psB = ctx.enter_context(tc.tile_pool(name="psB", bufs=4, space="PSUM"))

        # ---- constants / weights ----
        bn1_sb = [consts.tile([128, 6], F32, tag=f"bn1_{ct}", name=f"bn1_{ct}")
                  for ct in range(2)]
        bn2_sb = [consts.tile([128, 2], F32, tag=f"bn2_{ct}", name=f"bn2_{ct}")
                  for ct in range(2)]
        for ct in range(2):
            nc.sync.dma_start(bn1_sb[ct][:], bn1_d.ap()[ct * 128:(ct + 1) * 128, :])
            nc.sync.dma_start(bn2_sb[ct][:], bn2_d.ap()[ct * 128:(ct + 1) * 128, :])

        eps_sb = consts.tile([128, 1], F32)
        nc.vector.memset(eps_sb[:], EPS)

        def x_chunks():
            # progressive chunk sizes: small first chunks so the h/projection
            # pipeline starts early, large later ones to limit DMA issue cost
            xc = 0
            for w in (512, 512, 1024, 2048):
                while xc < n:
                    yield xc, min(w, n - xc)
                    xc += w
                    break
            while xc < n:
                yield xc, min(2048, n - xc)
                xc += 2048

        for _rep in range(reps):
            # ---- x load first (DMA issue slots are serial on SP) ----
            x_sb = [bigA.tile([128, n], F32, tag="bigA", name=f"x_{i}")
                    for i in range(2)]
            for ct in range(2):
                for xc, w in x_chunks():
                    nc.sync.dma_start(
                        x_sb[ct][:, xc:xc + w],
                        x_d.ap()[ct * 128:(ct + 1) * 128, xc:xc + w],
                    )

            if _rep == 0:
                w_r = {}
                for nm in ("wqT", "wkT", "wvT", "woT"):
                    w_r[nm] = []
                    for ct in range(2):
                        raw = wraw.tile([128, C], F32, tag="wld", name="wld")
                        nc.sync.dma_start(raw[:],
                                          w_d[nm].ap()[ct * 128:(ct + 1) * 128, :])
                        wr = consts.tile([128, C], F32R, tag=f"{nm}_{ct}",
                                         name=f"{nm}r_{ct}")
                        nc.vector.tensor_copy(wr[:], raw[:])
                        w_r[nm].append(wr)

            def make_h(scol, tcol):
                hs = []
                for ct in range(2):
                    h = bigB.tile([128, n], F32R, tag="bigB", name=f"h_{ct}")
                    for xc, w in x_chunks():
                        nc.scalar.activation(
                            h[:, xc:xc + w], x_sb[ct][:, xc:xc + w], AF.Relu,
                            bias=bn1_sb[ct][:, tcol:tcol + 1],
                            scale=bn1_sb[ct][:, scol:scol + 1],
                        )
                    hs.append(h)
                return hs

            # ---- projections ----
            def proj_cn(wt, hs, pool, tag):
                res = [pool.tile([128, n], F32R, tag=tag, name=f"{tag}p_{i}")
                       for i in range(2)]
                for co in range(2):
                    for nt in range(n // 512):
                        ps = psA.tile([128, 512], F32, tag="ps", name="ps")
                        for ci in range(2):
                            nc.tensor.matmul(
                                ps[:],
                                wt[ci][:, co * 128:(co + 1) * 128],
                                hs[ci][:, nt * 512:(nt + 1) * 512],
                                start=(ci == 0), stop=(ci == 1),
                            )
                        nc.vector.tensor_copy(
                            res[co][:, nt * 512:(nt + 1) * 512], ps[:])
                return res

            if shared_h:
                h_all = make_h(0, 1)
                h_q = h_k = h_v = h_all
            else:
                h_q = make_h(0, 1)
            q_r = proj_cn(w_r["wqT"], h_q, bigC, "bigC")
            if not shared_h:
                h_k = make_h(2, 3)
            k_r = proj_cn(w_r["wkT"], h_k, bigD, "bigD")
            if not shared_h:
                h_v = make_h(4, 5)

            # v in [n, c] layout: [128, NK, 256]
            v_r = bigE.tile([128, NK, C], F32R, tag="v", name="v_r")
            for kt in range(NK):
                ps = psA.tile([128, C], F32, tag="ps", name="ps")
                for ci in range(2):
                    nc.tensor.matmul(
                        ps[:],
                        h_v[ci][:, kt * 128:(kt + 1) * 128],
                        w_r["wvT"][ci][:, 0:C],
                        start=(ci == 0), stop=(ci == 1),
                    )
                nc.scalar.copy(v_r[:, kt, :], ps[:])

            # ---- attention, BN2 stat partials folded into the loop ----
            rT = [bigA.tile([128, n], F32, tag="bigA", name=f"rT_{i}")
                  for i in range(2)]
            s1part = [statp.tile([128, NQ], F32, tag=f"s1p{ct}", name=f"s1p_{ct}")
                      for ct in range(2)]
            s2part = [statp.tile([128, NQ], F32, tag=f"s2p{ct}", name=f"s2p_{ct}")
                      for ct in range(2)]

            def finalize_nq(nq, den, rt_ps):
                qs = slice(nq * 512, (nq + 1) * 512)
                dsum = smalls.tile([128, 512], F32, tag="dsum", name="dsum")
                nc.gpsimd.partition_all_reduce(dsum[:], den[:], 128,
                                               bass_isa.ReduceOp.add)
                rb = smalls.tile([128, 512], F32, tag="rb", name="rb")
                nc.vector.reciprocal_approx_fast(rb[:], dsum[:])
                for co in range(2):
                    nc.vector.tensor_mul(rT[co][:, qs], rt_ps[co][:], rb[:])
                    nc.vector.reduce_sum(s1part[co][:, nq:nq + 1], rT[co][:, qs],
                                         axis=AX.X)

            def emit_squares(nq):
                # BN2 sumsq partials on ACT — emitted mid-way through the NEXT
                # tile's loop so they never block the exp stream (strict FIFO)
                qs = slice(nq * 512, (nq + 1) * 512)
                for co in range(2):
                    scr = smalls.tile([128, 512], F32, tag="sqscr", name="sqscr")
                    nc.scalar.activation(
                        scr[:], rT[co][:, qs], AF.Square,
                        accum_out=s2part[co][:, nq:nq + 1],
                    )

            ar1_cols = [0]

            def emit_ar1(ncols):
                # stats over the first `ncols` tiles, AllReduced while the
                # last tiles' attention still computes -> the collective
                # latency + cross-core skew hide under compute; only AR2
                # (the remaining tiles, cores now aligned) is exposed
                ar1_cols[0] = ncols
                stats_a = statp.tile([128, 4], F32, tag="stats_a", name="stats_a")
                for ct in range(2):
                    nc.vector.reduce_sum(stats_a[:, 2 * ct:2 * ct + 1],
                                         s1part[ct][:, 0:ncols], axis=AX.X)
                    nc.vector.reduce_sum(stats_a[:, 2 * ct + 1:2 * ct + 2],
                                         s2part[ct][:, 0:ncols], axis=AX.X)
                nc.sync.dma_start(cc_in_a.ap(), stats_a[:])
                if fake_cc:
                    nc.sync.dma_start(cc_out_a.ap(), cc_in_a.ap())
                else:
                    nc.gpsimd.collective_compute(
                        "AllReduce",
                        mybir.AluOpType.add,
                        replica_groups=[list(range(NCORES))],
                        ins=[cc_in_a.ap().opt()],
                        outs=[cc_out_a.ap().opt()],
                    )

            pending = None
            for nq in range(NQ):
                qs = slice(nq * 512, (nq + 1) * 512)
                den = smalls.tile([128, 512], F32, tag="den", name="den")
                rt_ps = [psB.tile([128, 512], F32, tag="rt", name=f"rt_ps_{i}")
                         for i in range(2)]
                aTs = {}

                def emit_scores(kt, qs=qs, aTs=aTs):
                    s_ps = psA.tile([128, 512], F32, tag="ps", name="s_ps")
                    for ci in range(2):
                        nc.tensor.matmul(
                            s_ps[:],
                            k_r[ci][:, kt * 128:(kt + 1) * 128],
                            q_r[ci][:, qs],
                            start=(ci == 0), stop=(ci == 1),
                        )
                    aT = attn.tile([128, 512], F32R, tag="aT", name="aT")
                    nc.scalar.activation(aT[:], s_ps[:], AF.Exp, scale=1.0 / 16.0)
                    aTs[kt] = aT

                def emit_av(kt, den=den, rt_ps=rt_ps, aTs=aTs):
                    aT = aTs.pop(kt)
                    if kt == 0:
                        nc.vector.tensor_copy(den[:], aT[:].bitcast(F32))
                    else:
                        nc.vector.tensor_add(den[:], den[:], aT[:].bitcast(F32))
                    for co in range(2):
                        nc.tensor.matmul(
                            rt_ps[co][:],
                            v_r[:, kt, co * 128:(co + 1) * 128],
                            aT[:],
                            start=(kt == 0), stop=(kt == NK - 1),
                        )

                # software pipeline: scores/exp one kt ahead of av/den; the
                # previous tile's denominator/evac work is deferred until this
                # tile's first matmuls are queued so PE never waits on it
                emit_scores(0)
                if pending is not None:
                    finalize_nq(*pending)
                for kt in range(1, NK):
                    emit_scores(kt)
                    emit_av(kt - 1)
                    if kt == min(10, NK - 2) and pending is not None:
                        emit_squares(pending[0])
                        if (NQ >= 3 and nq == NQ - 2) or (NQ == 2 and nq == 1):
                            emit_ar1(NQ - 2 if NQ >= 3 else 1)
                emit_av(NK - 1)
                pending = (nq, den, rt_ps)
            finalize_nq(*pending)
            emit_squares(pending[0])

            # ---- BN2 stats: AR2 for the last tile (AR1 already in flight) ----
            stats_b = statp.tile([128, 4], F32, tag="stats_b", name="stats_b")
            c0 = ar1_cols[0] if NQ >= 2 else 0
            for ct in range(2):
                if NQ - c0 == 1:
                    nc.vector.tensor_copy(stats_b[:, 2 * ct:2 * ct + 1],
                                          s1part[ct][:, c0:NQ])
                    nc.vector.tensor_copy(stats_b[:, 2 * ct + 1:2 * ct + 2],
                                          s2part[ct][:, c0:NQ])
                else:
                    nc.vector.reduce_sum(stats_b[:, 2 * ct:2 * ct + 1],
                                         s1part[ct][:, c0:NQ], axis=AX.X)
                    nc.vector.reduce_sum(stats_b[:, 2 * ct + 1:2 * ct + 2],
                                         s2part[ct][:, c0:NQ], axis=AX.X)
            nc.sync.dma_start(cc_in_b.ap(), stats_b[:])

            # prefetch x for the residual while the collective runs
            x2_sb = [bigB.tile([128, n], F32, tag="bigB", name=f"x2_{i}")
                     for i in range(2)]
            for ct in range(2):
                nc.sync.dma_start(x2_sb[ct][:],
                                  x_d.ap()[ct * 128:(ct + 1) * 128, :])

            if fake_cc:
                nc.sync.dma_start(cc_out_b.ap(), cc_in_b.ap())
            else:
                nc.gpsimd.collective_compute(
                    "AllReduce",
                    mybir.AluOpType.add,
                    replica_groups=[list(range(NCORES))],
                    ins=[cc_in_b.ap().opt()],
                    outs=[cc_out_b.ap().opt()],
                )

            g_a = statp.tile([128, 4], F32, tag="ga", name="g_a")
            g_b = statp.tile([128, 4], F32, tag="gb", name="g_b")
            if NQ >= 2:
                nc.sync.dma_start(g_a[:], cc_out_a.ap())
            nc.sync.dma_start(g_b[:], cc_out_b.ap())
            g_sb = statp.tile([128, 4], F32, tag="g", name="g_sb")
            if NQ >= 2:
                nc.vector.tensor_add(g_sb[:], g_a[:], g_b[:])
            else:
                nc.vector.tensor_copy(g_sb[:], g_b[:])

            so_t, to_t = [], []
            for ct in range(2):
                mean = statp.tile([128, 1], F32, tag=f"mean{ct}", name=f"mean{ct}")
                nc.vector.tensor_scalar_mul(mean[:], g_sb[:, 2 * ct:2 * ct + 1],
                                            1.0 / count)
                ex2 = statp.tile([128, 1], F32, tag=f"ex2{ct}", name=f"ex2{ct}")
                nc.vector.tensor_scalar_mul(ex2[:], g_sb[:, 2 * ct + 1:2 * ct + 2],
                                            1.0 / count)
                m2 = statp.tile([128, 1], F32, tag=f"m2{ct}", name=f"m2{ct}")
                nc.vector.tensor_mul(m2[:], mean[:], mean[:])
                var = statp.tile([128, 1], F32, tag=f"var{ct}", name=f"var{ct}")
                nc.vector.tensor_sub(var[:], ex2[:], m2[:])
                std = statp.tile([128, 1], F32, tag=f"std{ct}", name=f"std{ct}")
                nc.scalar.activation(std[:], var[:], AF.Sqrt, bias=eps_sb[:])
                inv = statp.tile([128, 1], F32, tag=f"inv{ct}", name=f"inv{ct}")
                nc.vector.reciprocal(inv[:], std[:])
                so = statp.tile([128, 1], F32, tag=f"so{ct}", name=f"so{ct}")
                nc.vector.tensor_mul(so[:], inv[:], bn2_sb[ct][:, 0:1])
                tmp = statp.tile([128, 1], F32, tag=f"tmp{ct}", name=f"tmp{ct}")
                nc.vector.tensor_mul(tmp[:], mean[:], so[:])
                to = statp.tile([128, 1], F32, tag=f"to{ct}", name=f"to{ct}")
                nc.vector.tensor_sub(to[:], bn2_sb[ct][:, 1:2], tmp[:])
                so_t.append(so)
                to_t.append(to)

            # ---- h_o = relu(so*rT + to) and out = x + wo @ h_o, per slice ----
            ho = [bigC.tile([128, n], F32R, tag="bigC", name=f"ho_{i}")
                  for i in range(2)]
            for nt in range(n // 512):
                ns_ = slice(nt * 512, (nt + 1) * 512)
                nc.scalar.activation(ho[0][:, ns_], rT[0][:, ns_], AF.Relu,
                                     bias=to_t[0][:], scale=so_t[0][:])
                nc.vector.tensor_scalar(ho[1][:, ns_], rT[1][:, ns_],
                                        scalar1=so_t[1][:], scalar2=to_t[1][:],
                                        op0=mybir.AluOpType.mult,
                                        op1=mybir.AluOpType.add)
                nc.vector.tensor_scalar_max(ho[1][:, ns_],
                                            ho[1][:, ns_].bitcast(F32), 0.0)
                for co in range(2):
                    ps = psA.tile([128, 512], F32, tag="ps", name="ps")
                    for ci in range(2):
                        nc.tensor.matmul(
                            ps[:],
                            w_r["woT"][ci][:, co * 128:(co + 1) * 128],
                            ho[ci][:, ns_],
                            start=(ci == 0), stop=(ci == 1),
                        )
                    y = outp.tile([128, 512], F32, tag="y", name="y")
                    nc.vector.tensor_add(y[:], ps[:], x2_sb[co][:, ns_])
                    nc.sync.dma_start(
                        out_d.ap()[co * 128:(co + 1) * 128, ns_],
                        y[:],
                    )

    return nc


_CACHE = {}


def _get_nc(shared_h: bool):
    if shared_h not in _CACHE:
        nc = bacc.Bacc(trn_type="TRN2", target_bir_lowering=False, debug=False,
                       num_devices=NCORES)
        _build(nc, shared_h)
        nc.compile()
        _CACHE[shared_h] = nc
    return _CACHE[shared_h]


def kernel(x, wq, wk, wv, wo, gq, bq, gk, bk, gv, bv, go, bo):
    x = np.asarray(x, dtype=np.float32)
    b, c, hh, ww = x.shape
    assert (b, c, hh * ww) == (NCORES, C, N), f"unexpected shape {x.shape}"

    in_maps, shared_h = _host_prep(
        x, np.asarray(wq), np.asarray(wk), np.asarray(wv), np.asarray(wo),
        np.asarray(gq), np.asarray(bq), np.asarray(gk), np.asarray(bk),
        np.asarray(gv), np.asarray(bv), np.asarray(go), np.asarray(bo))

    nc = _get_nc(shared_h)
    res = run_bass_kernel_spmd(nc, in_maps, core_ids=list(range(NCORES)))
    out = np.stack([res.results[i]["out"] for i in range(NCORES)], axis=0)
    return out.reshape(b, c, hh, ww).astype(np.float32)



# revision 1
# speedup vs baseline: 1.0260x; 1.0260x over previous
"""TRN2 Bass kernel for nn_Attention_69655779606628 (8-core SPMD).

BN+ReLU / QKV self-attention / softmax / BN+ReLU / residual.

Sharding: data-parallel over batch b=8 -> one batch item per NeuronCore; the
small [256,256] weights and BN params are replicated. BN1 batch stats are
computed on host (x is fully known there and folds to a per-channel affine);
BN2 batch stats (over the attention output) are computed on device and
synchronized across the 8 cores with AllReduces (exact sync-BN semantics) --
split into two collectives so the first one's latency and cross-core skew
hide under the last attention tiles' compute.

All matmuls run as float32r (full-rate fp32 mode of the PE array); softmax is
computed in transposed [k,q] layout so no transposes are needed anywhere.
"""

import sys

for _p in ("/opt/trn_rl_repo", "/root/.axon_site/_ro/trn_rl_repo"):
    if _p not in sys.path:
        sys.path.insert(0, _p)

import numpy as np
from contextlib import ExitStack

import concourse.bass as bass
import concourse.bass_isa as bass_isa
import concourse.mybir as mybir
import concourse.tile as tile
from concourse import bacc
from concourse.bass_utils import run_bass_kernel_spmd

F32 = mybir.dt.float32
F32R = mybir.dt.float32r
AF = mybir.ActivationFunctionType
AX = mybir.AxisListType

EPS = 1e-5
NCORES = 8
C = 256
N = 4096  # h*w = 64*64

def _host_prep(x_all, wq, wk, wv, wo, gq, bq, gk, bk, gv, bv, go, bo):
    """Host-side prep: BN1 stats + per-core input maps."""
    b = x_all.shape[0]
    assert b == NCORES
    xv = x_all.reshape(b, C, N)

    x64 = xv.astype(np.float64)
    mean = x64.mean(axis=(0, 2))
    var = ((x64 - mean[None, :, None]) ** 2).mean(axis=(0, 2))
    inv = 1.0 / np.sqrt(var + EPS)

    def fold(g, bb):
        s = g.astype(np.float64) * inv
        t = bb.astype(np.float64) - mean * s
        return s.astype(np.float32), t.astype(np.float32)

    sq, tq = fold(gq, bq)
    sk, tk = fold(gk, bk)
    sv, tv = fold(gv, bv)
    shared_h = (
        np.allclose(sq, sk) and np.allclose(sq, sv)
        and np.allclose(tq, tk) and np.allclose(tq, tv)
    )

    bn1 = np.stack([sq, tq, sk, tk, sv, tv], axis=1).astype(np.float32)  # [256, 6]
    bn2 = np.stack([go, bo], axis=1).astype(np.float32)  # [256, 2]

    common = {
        "wqT": np.ascontiguousarray(wq.T).astype(np.float32),
        "wkT": np.ascontiguousarray(wk.T).astype(np.float32),
        "wvT": np.ascontiguousarray(wv.T).astype(np.float32),
        "woT": np.ascontiguousarray(wo.T).astype(np.float32),
        "bn1": bn1,
        "bn2": bn2,
    }
    in_maps = [
        {"x": np.ascontiguousarray(xv[i]), **common} for i in range(NCORES)
    ]
    return in_maps, shared_h


def _build(nc: bass.Bass, shared_h: bool):
    n = N
    fake_cc = False
    reps = 1
    assert n % 512 == 0
    NQ = n // 512   # query tiles (512 wide)
    NK = n // 128   # key tiles (128 wide)
    count = float(NCORES * n)  # BN2 element count per channel

    x_d = nc.dram_tensor("x", [C, n], F32, kind="ExternalInput")
    w_d = {
        nm: nc.dram_tensor(nm, [C, C], F32, kind="ExternalInput")
        for nm in ("wqT", "wkT", "wvT", "woT")
    }
    bn1_d = nc.dram_tensor("bn1", [C, 6], F32, kind="ExternalInput")
    bn2_d = nc.dram_tensor("bn2", [C, 2], F32, kind="ExternalInput")
    out_d = nc.dram_tensor("out", [C, n], F32, kind="ExternalOutput")
    cc_in_a = nc.dram_tensor("cc_in_a", [128, 4], F32)
    cc_out_a = nc.dram_tensor("cc_out_a", [128, 4], F32, addr_space="Shared")
    cc_in_b = nc.dram_tensor("cc_in_b", [128, 4], F32)
    cc_out_b = nc.dram_tensor("cc_out_b", [128, 4], F32, addr_space="Shared")

    with tile.TileContext(nc) as tc, ExitStack() as ctx:
        consts = ctx.enter_context(tc.tile_pool(name="consts", bufs=1))
        wraw = ctx.enter_context(tc.tile_pool(name="wraw", bufs=2))
        bigA = ctx.enter_context(tc.tile_pool(name="bigA", bufs=2))  # x -> rT
        bigB = ctx.enter_context(tc.tile_pool(name="bigB", bufs=2))  # h -> x2
        bigC = ctx.enter_context(tc.tile_pool(name="bigC", bufs=2))  # q -> ho
        bigD = ctx.enter_context(tc.tile_pool(name="bigD", bufs=2))  # k
        bigE = ctx.enter_context(tc.tile_pool(name="bigE", bufs=1))  # v
        attn = ctx.enter_context(tc.tile_pool(name="attn", bufs=4))
        smalls = ctx.enter_context(tc.tile_pool(name="smalls", bufs=2))
        statp = ctx.enter_context(tc.tile_pool(name="statp", bufs=1))
        outp = ctx.enter_context(tc.tile_pool(name="outp", bufs=2))
        psA = ctx.enter_context(tc.tile_pool(name="psA", bufs=4, space="PSUM"))
        psB = ctx.enter_context(tc.tile_pool(name="psB", bufs=4, space="PSUM"))

        # ---- constants / weights ----
        bn1_sb = [consts.tile([128, 6], F32, tag=f"bn1_{ct}", name=f"bn1_{ct}")
                  for ct in range(2)]
        bn2_sb = [consts.tile([128, 2], F32, tag=f"bn2_{ct}", name=f"bn2_{ct}")
                  for ct in range(2)]
        for ct in range(2):
            nc.sync.dma_start(bn1_sb[ct][:], bn1_d.ap()[ct * 128:(ct + 1) * 128, :])
            nc.sync.dma_start(bn2_sb[ct][:], bn2_d.ap()[ct * 128:(ct + 1) * 128, :])

        eps_sb = consts.tile([128, 1], F32)
        nc.vector.memset(eps_sb[:], EPS)

        def x_chunks():
            # progressive chunk sizes: small first chunks so the h/projection
            # pipeline starts early, large later ones to limit DMA issue cost
            xc = 0
            for w in (512, 512, 1024, 2048):
                while xc < n:
                    yield xc, min(w, n - xc)
                    xc += w
                    break
            while xc < n:
                yield xc, min(2048, n - xc)
                xc += 2048

        for _rep in range(reps):
            # ---- x load first (DMA issue slots are serial on SP) ----
            x_sb = [bigA.tile([128, n], F32, tag="bigA", name=f"x_{i}")
                    for i in range(2)]
            for ct in range(2):
                for xc, w in x_chunks():
                    nc.sync.dma_start(
                        x_sb[ct][:, xc:xc + w],
                        x_d.ap()[ct * 128:(ct + 1) * 128, xc:xc + w],
                    )

            if _rep == 0:
                w_r = {}
                for nm in ("wqT", "wkT", "wvT", "woT"):
                    w_r[nm] = []
                    for ct in range(2):
                        raw = wraw.tile([128, C], F32, tag="wld", name="wld")
                        nc.sync.dma_start(raw[:],
                                          w_d[nm].ap()[ct * 128:(ct + 1) * 128, :])
                        wr = consts.tile([128, C], F32R, tag=f"{nm}_{ct}",
                                         name=f"{nm}r_{ct}")
                        nc.vector.tensor_copy(wr[:], raw[:])
                        w_r[nm].append(wr)

            def make_h(scol, tcol):
                hs = []
                for ct in range(2):
                    h = bigB.tile([128, n], F32R, tag="bigB", name=f"h_{ct}")
                    for xc, w in x_chunks():
                        nc.scalar.activation(
                            h[:, xc:xc + w], x_sb[ct][:, xc:xc + w], AF.Relu,
                            bias=bn1_sb[ct][:, tcol:tcol + 1],
                            scale=bn1_sb[ct][:, scol:scol + 1],
                        )
                    hs.append(h)
                return hs

            # ---- projections ----
            def proj_cn(wt, hs, pool, tag):
                res = [pool.tile([128, n], F32R, tag=tag, name=f"{tag}p_{i}")
                       for i in range(2)]
                for co in range(2):
                    for nt in range(n // 512):
                        ps = psA.tile([128, 512], F32, tag="ps", name="ps")
                        for ci in range(2):
                            nc.tensor.matmul(
                                ps[:],
                                wt[ci][:, co * 128:(co + 1) * 128],
                                hs[ci][:, nt * 512:(nt + 1) * 512],
                                start=(ci == 0), stop=(ci == 1),
                            )
                        nc.vector.tensor_copy(
                            res[co][:, nt * 512:(nt + 1) * 512], ps[:])
                return res

            if shared_h:
                h_all = make_h(0, 1)
                h_q = h_k = h_v = h_all
            else:
                h_q = make_h(0, 1)
            q_r = proj_cn(w_r["wqT"], h_q, bigC, "bigC")
            if not shared_h:
                h_k = make_h(2, 3)
            k_r = proj_cn(w_r["wkT"], h_k, bigD, "bigD")
            if not shared_h:
                h_v = make_h(4, 5)

            # v in [n, c] layout: [128, NK, 256]
            v_r = bigE.tile([128, NK, C], F32R, tag="v", name="v_r")
            for kt in range(NK):
                ps = psA.tile([128, C], F32, tag="ps", name="ps")
                for ci in range(2):
                    nc.tensor.matmul(
                        ps[:],
                        h_v[ci][:, kt * 128:(kt + 1) * 128],
                        w_r["wvT"][ci][:, 0:C],
                        start=(ci == 0), stop=(ci == 1),
                    )
                nc.scalar.copy(v_r[:, kt, :], ps[:])

            # ---- attention, BN2 stat partials folded into the loop ----
            rT = [bigA.tile([128, n], F32, tag="bigA", name=f"rT_{i}")
                  for i in range(2)]
            s1part = [statp.tile([128, NQ], F32, tag=f"s1p{ct}", name=f"s1p_{ct}")
                      for ct in range(2)]
            s2part = [statp.tile([128, NQ], F32, tag=f"s2p{ct}", name=f"s2p_{ct}")
                      for ct in range(2)]

            def finalize_nq(nq, den, rt_ps):
                qs = slice(nq * 512, (nq + 1) * 512)
                dsum = smalls.tile([128, 512], F32, tag="dsum", name="dsum")
                nc.gpsimd.partition_all_reduce(dsum[:], den[:], 128,
                                               bass_isa.ReduceOp.add)
                rb = smalls.tile([128, 512], F32, tag="rb", name="rb")
                nc.vector.reciprocal_approx_fast(rb[:], dsum[:])
                for co in range(2):
                    nc.vector.tensor_mul(rT[co][:, qs], rt_ps[co][:], rb[:])
                    nc.vector.reduce_sum(s1part[co][:, nq:nq + 1], rT[co][:, qs],
                                         axis=AX.X)

            def emit_squares(nq):
                # BN2 sumsq partials on ACT — emitted mid-way through the NEXT
                # tile's loop so they never block the exp stream (strict FIFO)
                qs = slice(nq * 512, (nq + 1) * 512)
                for co in range(2):
                    scr = smalls.tile([128, 512], F32, tag="sqscr", name="sqscr")
                    nc.scalar.activation(
                        scr[:], rT[co][:, qs], AF.Square,
                        accum_out=s2part[co][:, nq:nq + 1],
                    )

            ar1_cols = [0]

            def emit_ar1(ncols):
                # stats over the first `ncols` tiles, AllReduced while the
                # last tiles' attention still computes -> the collective
                # latency + cross-core skew hide under compute; only AR2
                # (the remaining tiles, cores now aligned) is exposed
                ar1_cols[0] = ncols
                stats_a = statp.tile([128, 4], F32, tag="stats_a", name="stats_a")
                for ct in range(2):
                    nc.vector.reduce_sum(stats_a[:, 2 * ct:2 * ct + 1],
                                         s1part[ct][:, 0:ncols], axis=AX.X)
                    nc.vector.reduce_sum(stats_a[:, 2 * ct + 1:2 * ct + 2],
                                         s2part[ct][:, 0:ncols], axis=AX.X)
                nc.sync.dma_start(cc_in_a.ap(), stats_a[:])
                if fake_cc:
                    nc.sync.dma_start(cc_out_a.ap(), cc_in_a.ap())
                else:
                    nc.gpsimd.collective_compute(
                        "AllReduce",
                        mybir.AluOpType.add,
                        replica_groups=[list(range(NCORES))],
                        ins=[cc_in_a.ap().opt()],
                        outs=[cc_out_a.ap().opt()],
                    )

            pending = None
            for nq in range(NQ):
                qs = slice(nq * 512, (nq + 1) * 512)
                den = smalls.tile([128, 512], F32, tag="den", name="den")
                rt_ps = [psB.tile([128, 512], F32, tag="rt", name=f"rt_ps_{i}")
                         for i in range(2)]
                aTs = {}

                def emit_scores(kt, qs=qs, aTs=aTs):
                    s_ps = psA.tile([128, 512], F32, tag="ps", name="s_ps")
                    for ci in range(2):
                        nc.tensor.matmul(
                            s_ps[:],
                            k_r[ci][:, kt * 128:(kt + 1) * 128],
                            q_r[ci][:, qs],
                            start=(ci == 0), stop=(ci == 1),
                        )
                    aT = attn.tile([128, 512], F32R, tag="aT", name="aT")
                    nc.scalar.activation(aT[:], s_ps[:], AF.Exp, scale=1.0 / 16.0)
                    aTs[kt] = aT

                def emit_av(kt, den=den, rt_ps=rt_ps, aTs=aTs):
                    aT = aTs.pop(kt)
                    if kt == 0:
                        nc.vector.tensor_copy(den[:], aT[:].bitcast(F32))
                    else:
                        nc.vector.tensor_add(den[:], den[:], aT[:].bitcast(F32))
                    for co in range(2):
                        nc.tensor.matmul(
                            rt_ps[co][:],
                            v_r[:, kt, co * 128:(co + 1) * 128],
                            aT[:],
                            start=(kt == 0), stop=(kt == NK - 1),
                        )

                # software pipeline: scores/exp one kt ahead of av/den; the
                # previous tile's denominator/evac work is deferred until this
                # tile's first matmuls are queued so PE never waits on it
                emit_scores(0)
                if pending is not None:
                    finalize_nq(*pending)
                for kt in range(1, NK):
                    emit_scores(kt)
                    emit_av(kt - 1)
                    if kt == min(10, NK - 2) and pending is not None:
                        emit_squares(pending[0])
                        if (NQ >= 3 and nq == NQ - 2) or (NQ == 2 and nq == 1):
                            emit_ar1(NQ - 2 if NQ >= 3 else 1)
                emit_av(NK - 1)
                pending = (nq, den, rt_ps)
            finalize_nq(*pending)
            emit_squares(pending[0])

            # ---- BN2 stats: AR2 for the last tile (AR1 already in flight) ----
            stats_b = statp.tile([128, 4], F32, tag="stats_b", name="stats_b")
            c0 = ar1_cols[0] if NQ >= 2 else 0
            for ct in range(2):
                if NQ - c0 == 1:
                    nc.vector.tensor_copy(stats_b[:, 2 * ct:2 * ct + 1],
                                          s1part[ct][:, c0:NQ])
                    nc.vector.tensor_copy(stats_b[:, 2 * ct + 1:2 * ct + 2],
                                          s2part[ct][:, c0:NQ])
                else:
                    nc.vector.reduce_sum(stats_b[:, 2 * ct:2 * ct + 1],
                                         s1part[ct][:, c0:NQ], axis=AX.X)
                    nc.vector.reduce_sum(stats_b[:, 2 * ct + 1:2 * ct + 2],
                                         s2part[ct][:, c0:NQ], axis=AX.X)
            nc.sync.dma_start(cc_in_b.ap(), stats_b[:])

            # prefetch x for the residual while the collective runs
            x2_sb = [bigB.tile([128, n], F32, tag="bigB", name=f"x2_{i}")
                     for i in range(2)]
            for ct in range(2):
                nc.sync.dma_start(x2_sb[ct][:],
                                  x_d.ap()[ct * 128:(ct + 1) * 128, :])

            if fake_cc:
                nc.sync.dma_start(cc_out_b.ap(), cc_in_b.ap())
            else:
                nc.gpsimd.collective_compute(
                    "AllReduce",
                    mybir.AluOpType.add,
                    replica_groups=[list(range(NCORES))],
                    ins=[cc_in_b.ap().opt()],
                    outs=[cc_out_b.ap().opt()],
                )

            g_a = statp.tile([128, 4], F32, tag="ga", name="g_a")
            g_b = statp.tile([128, 4], F32, tag="gb", name="g_b")
            if NQ >= 2:
                nc.sync.dma_start(g_a[:], cc_out_a.ap())
            nc.sync.dma_start(g_b[:], cc_out_b.ap())
            g_sb = statp.tile([128, 4], F32, tag="g", name="g_sb")
            if NQ >= 2:
                nc.vector.tensor_add(g_sb[:], g_a[:], g_b[:])
            else:
                nc.vector.tensor_copy(g_sb[:], g_b[:])

            so_t, to_t = [], []
            for ct in range(2):
                mean = statp.tile([128, 1], F32, tag=f"mean{ct}", name=f"mean{ct}")
                nc.vector.tensor_scalar_mul(mean[:], g_sb[:, 2 * ct:2 * ct + 1],
                                            1.0 / count)
                ex2 = statp.tile([128, 1], F32, tag=f"ex2{ct}", name=f"ex2{ct}")
                nc.vector.tensor_scalar_mul(ex2[:], g_sb[:, 2 * ct + 1:2 * ct + 2],
                                            1.0 / count)
                m2 = statp.tile([128, 1], F32, tag=f"m2{ct}", name=f"m2{ct}")
                nc.vector.tensor_mul(m2[:], mean[:], mean[:])
                var = statp.tile([128, 1], F32, tag=f"var{ct}", name=f"var{ct}")
                nc.vector.tensor_sub(var[:], ex2[:], m2[:])
                std = statp.tile([128, 1], F32, tag=f"std{ct}", name=f"std{ct}")
                nc.scalar.activation(std[:], var[:], AF.Sqrt, bias=eps_sb[:])
                inv = statp.tile([128, 1], F32, tag=f"inv{ct}", name=f"inv{ct}")
                nc.vector.reciprocal(inv[:], std[:])
                so = statp.tile([128, 1], F32, tag=f"so{ct}", name=f"so{ct}")
                nc.vector.tensor_mul(so[:], inv[:], bn2_sb[ct][:, 0:1])
                tmp = statp.tile([128, 1], F32, tag=f"tmp{ct}", name=f"tmp{ct}")
                nc.vector.tensor_mul(tmp[:], mean[:], so[:])
                to = statp.tile([128, 1], F32, tag=f"to{ct}", name=f"to{ct}")
                nc.vector.tensor_sub(to[:], bn2_sb[ct][:, 1:2], tmp[:])
                so_t.append(so)
                to_t.append(to)

            # ---- h_o = relu(so*rT + to) and out = x + wo @ h_o, per slice ----
            ho = [bigC.tile([128, n], F32R, tag="bigC", name=f"ho_{i}")
                  for i in range(2)]
            for nt in range(n // 512):
                ns_ = slice(nt * 512, (nt + 1) * 512)
                nc.scalar.activation(ho[0][:, ns_], rT[0][:, ns_], AF.Relu,
                                     bias=to_t[0][:], scale=so_t[0][:])
                nc.vector.tensor_scalar(ho[1][:, ns_], rT[1][:, ns_],
                                        scalar1=so_t[1][:], scalar2=to_t[1][:],
                                        op0=mybir.AluOpType.mult,
                                        op1=mybir.AluOpType.add)
                nc.vector.tensor_scalar_max(ho[1][:, ns_],
                                            ho[1][:, ns_].bitcast(F32), 0.0)
                for co in range(2):
                    ps = psA.tile([128, 512], F32, tag="ps", name="ps")
                    for ci in range(2):
                        nc.tensor.matmul(
                            ps[:],
                            w_r["woT"][ci][:, co * 128:(co + 1) * 128],
                            ho[ci][:, ns_],
                            start=(ci == 0), stop=(ci == 1),
                        )
                    y = outp.tile([128, 512], F32, tag="y", name="y")
                    nc.vector.tensor_add(y[:], ps[:], x2_sb[co][:, ns_])
                    nc.sync.dma_start(
                        out_d.ap()[co * 128:(co + 1) * 128, ns_],
                        y[:],
                    )

    return nc


_CACHE = {}


def _get_nc(shared_h: bool):
    if shared_h not in _CACHE:
        nc = bacc.Bacc(trn_type="TRN2", target_bir_lowering=False, debug=False,
                       num_devices=NCORES)
        _build(nc, shared_h)
        nc.compile()
        _CACHE[shared_h] = nc
    return _CACHE[shared_h]


def kernel(x, wq, wk, wv, wo, gq, bq, gk, bk, gv, bv, go, bo):
    x = np.asarray(x, dtype=np.float32)
    b, c, hh, ww = x.shape
    assert (b, c, hh * ww) == (NCORES, C, N), f"unexpected shape {x.shape}"

    in_maps, shared_h = _host_prep(
        x, np.asarray(wq), np.asarray(wk), np.asarray(wv), np.asarray(wo),
        np.asarray(gq), np.asarray(bq), np.asarray(gk), np.asarray(bk),
        np.asarray(gv), np.asarray(bv), np.asarray(go), np.asarray(bo))

    nc = _get_nc(shared_h)
    res = run_bass_kernel_spmd(nc, in_maps, core_ids=list(range(NCORES)))
    out = np.stack([res.results[i]["out"] for i in range(NCORES)], axis=0)
    return out.reshape(b, c, hh, ww).astype(np.float32)

